# revision 12
# baseline (speedup 1.0000x reference)
"""ClusterISAAttention Trainium2 kernel (8 NeuronCores, SPMD).

Strategy
--------
Host: per batch (2), stable-sort queries by window id, split into 4
contiguous quarters of 2048 queries -> 8 (batch, quarter) shards, one
per core.  Each quarter's queries are grouped into "slots": one window
per slot, <=32 queries per slot (windows with more queries span several
slots; every slot carries its own copy of the window's 49 x_permute
rows).  Slot count is padded to a fixed 128 so the SPMD program is
static; padded queries/slots are dummies whose outputs are discarded.

Device (per core): everything is matmul-friendly bf16 (fp32 psum):
  qT  = (Wq @ xqT + bq)*scale          [256, 4096]   dims-major
  kT  -> evicted into block-diagonal per-slot tables kbd[g][128, slot, 256]
         (4 heads per group g; head block-diag so one matmul handles 4 heads)
  vT  -> per-phase [128, 64k, 32s] tables, xbar-DMA-transposed into
         block-diag vbd[pair][128(2x64 keys), slot, 64(2x32 dims)]
  S   = qT_slot.T @ kbd  -> psum [32q, 512] per 4-slot tile (query-major)
  A   = exp(S) (no max-subtraction: logits are O(1) by construction),
        normalized by 1/rowsum via DVE, pads zeroed
  AT  = xbar-DMA transpose of A -> atbd[pair][128(2x64 keys), 32q]
  ctxT= vbd.T @ atbd -> [64(2x32 dims), 32q] psum -> bf16 [256, 4096]
  outT= Wo @ ctxT + bo -> fp32 [256, 4096] -> DRAM
Host: outT columns scattered back to the original query order.
"""

import os
import sys
import numpy as np
import ml_dtypes

for _p in ("/opt/trn_rl_repo", "/root/.axon_site/_ro/trn_rl_repo"):
    if os.path.isdir(_p) and _p not in sys.path:
        sys.path.append(_p)

import concourse.bass as bass
import concourse.tile as tile
from concourse import bacc
from concourse import mybir

F32 = mybir.dt.float32
BF16 = mybir.dt.bfloat16
AF = mybir.ActivationFunctionType
ALU = mybir.AluOpType
AX = mybir.AxisListType

# problem constants (hardcoded per contest rules)
B, N, C, H, HD, W, K = 2, 8192, 256, 8, 32, 361, 49
SCALE = float(HD) ** -0.5

NCORES = 8
QTRS = 4                 # query quarters per batch
NLOC = N // QTRS         # 2048 queries per core
CAP = 32                 # queries per slot
NSLOT = 128              # fixed slot count per core (>= ~95 needed)
NQ = NSLOT * CAP         # 4096 padded queries
PH = 4                   # phases
SPP = NSLOT // PH        # 32 slots per phase
TPP = SPP // 4           # 8 four-slot tiles per phase
KP = 64                  # padded keys per head block

DEBUG_DUMP = False       # add DRAM dumps of intermediates (debug builds only)


def _build_program():
    nc = bacc.Bacc("TRN2", target_bir_lowering=False, debug=False,
                   num_devices=NCORES)

    xqT = nc.declare_dram_parameter("xqT", [C, NQ], BF16, isOutput=False).ap()
    xpT = nc.declare_dram_parameter("xpT", [C, NSLOT * K], BF16, isOutput=False).ap()
    wts = {
        nm: nc.declare_dram_parameter(f"w{nm}T", [C, C], BF16, isOutput=False).ap()
        for nm in ("q", "k", "v", "o")
    }
    bss = {
        nm: nc.declare_dram_parameter(f"b{nm}", [C], F32, isOutput=False).ap()
        for nm in ("q", "k", "v", "o")
    }
    outT = nc.declare_dram_parameter("outT", [C, NQ], F32, isOutput=True).ap()
    dbg = None
    if DEBUG_DUMP:
        dbg = {
            "d_qT": nc.declare_dram_parameter("d_qT", [2, 128, NQ], F32, isOutput=True).ap(),
            "d_kbd": nc.declare_dram_parameter("d_kbd", [2, 128, SPP * 4 * KP], F32, isOutput=True).ap(),
            "d_vT": nc.declare_dram_parameter("d_vT", [2, 128, SPP * 2 * KP], F32, isOutput=True).ap(),
            "d_vbd": nc.declare_dram_parameter("d_vbd", [4, 128, SPP * 2 * CAP], F32, isOutput=True).ap(),
            "d_a": nc.declare_dram_parameter("d_a", [128, 4 * TPP * 2 * KP], F32, isOutput=True).ap(),
            "d_atbd": nc.declare_dram_parameter("d_atbd", [4, 128, TPP * 4 * CAP], F32, isOutput=True).ap(),
            "d_ctxT": nc.declare_dram_parameter("d_ctxT", [2, 128, NQ], F32, isOutput=True).ap(),
        }

    with tile.TileContext(nc) as tc:
        _kernel_body(tc, xqT, xpT, wts, bss, outT, dbg)
    nc.compile()
    return nc


def _kernel_body(tc, xqT, xpT, wts, bss, outT, dbg=None):
    from contextlib import ExitStack

    nc = tc.nc
    ctx = ExitStack()
    with ctx:
        singles = ctx.enter_context(tc.tile_pool(name="singles", bufs=1))
        pp = ctx.enter_context(tc.tile_pool(name="proj_ps", bufs=2, space="PSUM"))
        sp = ctx.enter_context(tc.tile_pool(name="s_ps", bufs=2, space="PSUM"))
        cp = ctx.enter_context(tc.tile_pool(name="ctx_ps", bufs=2, space="PSUM"))
        ostage = ctx.enter_context(tc.tile_pool(name="ostage", bufs=2))

        # ---- persistent SBUF tensors ----
        w_sb = {nm: singles.tile([128, 2, C], BF16, tag=f"w_{nm}", name=f"w_{nm}") for nm in wts}
        b_sb = {nm: singles.tile([128, 2], F32, tag=f"b_{nm}", name=f"b_{nm}") for nm in bss}
        xq_sb = [singles.tile([128, NQ], BF16, tag=f"xq{c}", name=f"xq{c}") for c in range(2)]
        xp_sb = [singles.tile([128, NSLOT * K], BF16, tag=f"xp{c}", name=f"xp{c}") for c in range(2)]
        qT_sb = [singles.tile([128, NQ], BF16, tag=f"qT{g}", name=f"qT{g}") for g in range(2)]
        kbd_sb = [singles.tile([128, SPP, 4 * KP], BF16, tag=f"kbd{g}", name=f"kbd{g}") for g in range(2)]
        vT_sb = [singles.tile([128, SPP, 2 * KP], BF16, tag=f"vT{m}", name=f"vT{m}") for m in range(2)]
        vbd_sb = [singles.tile([128, SPP, 2 * CAP], BF16, tag=f"vbd{p}", name=f"vbd{p}") for p in range(4)]
        a_sb = singles.tile([128, 4, TPP, 2 * KP], BF16, tag="a_sb", name="a_sb")
        den_sb = singles.tile([128, TPP, 8], F32, tag="den", name="den")
        rec_sb = singles.tile([128, TPP, 8], F32, tag="rec", name="rec")
        atbd_sb = [singles.tile([128, TPP, 4, CAP], BF16, tag=f"atbd{p}", name=f"atbd{p}") for p in range(4)]
        ctxT_sb = [singles.tile([128, NQ], BF16, tag=f"ctxT{c}", name=f"ctxT{c}") for c in range(2)]

        # ---- load inputs ----
        for nm in wts:
            nc.sync.dma_start(
                out=w_sb[nm][:], in_=wts[nm].rearrange("(s p) m -> p s m", p=128))
        for nm in bss:
            nc.sync.dma_start(
                out=b_sb[nm][:], in_=bss[nm].rearrange("(s p) -> p s", p=128))
        for c in range(2):
            nc.sync.dma_start(out=xq_sb[c][:], in_=xqT[c * 128:(c + 1) * 128, :])
            nc.sync.dma_start(out=xp_sb[c][:], in_=xpT[c * 128:(c + 1) * 128, :])

        # zero the never-written gaps once; they persist across phases
        for g in range(2):
            nc.vector.memset(kbd_sb[g][:], 0.0)
        for m in range(2):
            nc.vector.memset(vT_sb[m][:], 0.0)

        # ---- q projection: qT[m*128+d, i] ----
        for nch in range(NQ // 512):
            for m in range(2):
                ps = pp.tile([128, 512], F32, tag="ps", name="ps")
                for c in range(2):
                    nc.tensor.matmul(
                        ps[:], w_sb["q"][:, c, m * 128:(m + 1) * 128],
                        xq_sb[c][:, nch * 512:(nch + 1) * 512],
                        start=(c == 0), stop=(c == 1))
                nc.scalar.activation(
                    qT_sb[m][:, nch * 512:(nch + 1) * 512], ps[:], AF.Identity,
                    bias=b_sb["q"][:, m:m + 1], scale=SCALE)

        # ---- phases ----
        for ph in range(PH):
            # k/v projections for this phase's 32 slots (49*32 = 1568 cols)
            for proj in ("k", "v"):
                for ch in range(4):  # 8 slots per chunk
                    col0 = ph * SPP * K + ch * 8 * K
                    for m in range(2):
                        ps = pp.tile([128, 512], F32, tag="ps", name="ps")
                        for c in range(2):
                            nc.tensor.matmul(
                                ps[:, 0:8 * K], w_sb[proj][:, c, m * 128:(m + 1) * 128],
                                xp_sb[c][:, col0:col0 + 8 * K],
                                start=(c == 0), stop=(c == 1))
                        if proj == "k":
                            for bb in range(4):
                                nc.scalar.activation(
                                    kbd_sb[m][32 * bb:32 * bb + 32,
                                              ch * 8:(ch + 1) * 8,
                                              KP * bb:KP * bb + K],
                                    ps[32 * bb:32 * bb + 32, 0:8 * K].rearrange(
                                        "p (s k) -> p s k", k=K),
                                    AF.Identity, bias=b_sb["k"][32 * bb:32 * bb + 32, m:m + 1])
                        else:
                            for hh in range(4):
                                ko = KP * (hh % 2)
                                nc.scalar.activation(
                                    vT_sb[m][32 * hh:32 * hh + 32,
                                             ch * 8:(ch + 1) * 8, ko:ko + K],
                                    ps[32 * hh:32 * hh + 32, 0:8 * K].rearrange(
                                        "p (s k) -> p s k", k=K),
                                    AF.Identity,
                                    bias=b_sb["v"][32 * hh:32 * hh + 32, m:m + 1])

            # vT -> block-diag vbd via xbar transpose.  The xbar writes
            # dst partition = (src element index) mod 128 across all 128
            # partitions, so each slot block is 128 wide in vT with the
            # key-half pre-offset by 64*(h%2); the complementary half is
            # zero and lands as the block-diag zeros of vbd.
            for p in range(4):
                for a in range(2):
                    h = 2 * p + a
                    m, hh = divmod(h, 4)
                    nc.sync.dma_start_transpose(
                        out=vbd_sb[p][:, :, CAP * a:CAP * a + CAP],
                        in_=vT_sb[m][32 * hh:32 * hh + 32, :, :])

            # logits + exp + rowsum per 4-slot tile
            for t in range(TPP):
                st = sp.tile([128, 512], F32, tag="st", name="st")
                for sl in range(4):
                    s_ph = t * 4 + sl
                    qcol = (ph * SPP + s_ph) * CAP
                    for g in range(2):
                        nc.tensor.matmul(
                            st[32 * sl:32 * sl + 32, 256 * g:256 * g + 256],
                            qT_sb[g][:, qcol:qcol + CAP],
                            kbd_sb[g][:, s_ph, :],
                            start=(g == 0), stop=(g == 1),
                            skip_group_check=True, tile_position=(0, 32 * sl))
                nc.scalar.activation(
                    a_sb[:, :, t, :], st[:].rearrange("p (x y) -> p x y", x=4),
                    AF.Exp)
                nc.vector.tensor_reduce(
                    out=den_sb[:, t, :].rearrange("p (x a) -> p x a", x=4),
                    in_=a_sb[:, :, t, :].rearrange("p x (a j) -> p x a j", a=2)[:, :, :, 0:K],
                    axis=AX.X, op=ALU.add)
            nc.vector.reciprocal(
                rec_sb[:].rearrange("p a b -> p (a b)"),
                den_sb[:].rearrange("p a b -> p (a b)"))
            for t in range(TPP):
                a4 = a_sb[:, :, t, :].rearrange("p x (a j) -> p x a j", a=2)
                r4 = rec_sb[:, t, :].rearrange("p (x a) -> p x a", x=4) \
                    .unsqueeze(3).broadcast_to([128, 4, 2, KP])
                nc.vector.tensor_tensor(out=a4, in0=a4, in1=r4, op=ALU.mult)
            for a in range(2):
                nc.vector.memset(a_sb[:, :, :, KP * a + K:KP * a + KP], 0.0)

            # A -> atbd via xbar transpose
            for cq in range(4):
                for p in range(4):
                    nc.sync.dma_start_transpose(
                        out=atbd_sb[p][:, :, cq, :],
                        in_=a_sb[32 * cq:32 * cq + 32, p, :, :])

            # ctx: per slot, 4 pair-matmuls
            for s_ph in range(SPP):
                t, cq = divmod(s_ph, 4)
                cps = [cp.tile([128, CAP], F32, tag="cps0", name="cps0"),
                       cp.tile([128, CAP], F32, tag="cps1", name="cps1")]
                for p in range(4):
                    nc.tensor.matmul(
                        cps[p // 2][64 * (p % 2):64 * (p % 2) + 64, :],
                        vbd_sb[p][:, s_ph, :],
                        atbd_sb[p][:, t, cq, :],
                        start=True, stop=True,
                        skip_group_check=True, tile_position=(0, 64 * (p % 2)))
                col = (ph * SPP + s_ph) * CAP
                for j in range(2):
                    nc.vector.tensor_copy(ctxT_sb[j][:, col:col + CAP], cps[j][:])

        if dbg is not None:
            dpool = ctx.enter_context(tc.tile_pool(name="dbgpool", bufs=2))
            def dump(dst, src_ap):
                stg = dpool.tile([128, 512], F32, name="dstg", tag="dstg")
                fs = src_ap.free_size()
                src2 = src_ap.rearrange("p a b -> p (a b)") if len(src_ap.shape) == 3 else (src_ap.rearrange("p a b c -> p (a b c)") if len(src_ap.shape) == 4 else src_ap)
                for c0 in range(0, fs, 512):
                    w = min(512, fs - c0)
                    stg = dpool.tile([128, 512], F32, name="dstg", tag="dstg")
                    nc.vector.tensor_copy(stg[:, 0:w], src2[:, c0:c0 + w])
                    nc.sync.dma_start(out=dst[:, c0:c0 + w], in_=stg[:, 0:w])
            for g in range(2):
                dump(dbg["d_qT"][g], qT_sb[g][:])
                dump(dbg["d_kbd"][g], kbd_sb[g][:])
                dump(dbg["d_vT"][g], vT_sb[g][:])
                dump(dbg["d_ctxT"][g], ctxT_sb[g][:])
            for p in range(4):
                dump(dbg["d_vbd"][p], vbd_sb[p][:])
                dump(dbg["d_atbd"][p], atbd_sb[p][:])
            dump(dbg["d_a"], a_sb[:])

        # ---- output projection ----
        for nch in range(NQ // 512):
            for m in range(2):
                ps = pp.tile([128, 512], F32, tag="ps", name="ps")
                for c in range(2):
                    nc.tensor.matmul(
                        ps[:], w_sb["o"][:, c, m * 128:(m + 1) * 128],
                        ctxT_sb[c][:, nch * 512:(nch + 1) * 512],
                        start=(c == 0), stop=(c == 1))
                ot = ostage.tile([128, 512], F32, tag="ot", name="ot")
                nc.scalar.activation(ot[:], ps[:], AF.Identity,
                                     bias=b_sb["o"][:, m:m + 1])
                nc.sync.dma_start(
                    out=outT[m * 128:(m + 1) * 128, nch * 512:(nch + 1) * 512],
                    in_=ot[:])


_PROGRAM = None


def _get_program():
    global _PROGRAM
    if _PROGRAM is None:
        _PROGRAM = _build_program()
    return _PROGRAM


def _pack_core(x_b, xp_b, qidx, wins):
    """Build one core's padded inputs. Returns (xqT, xpT, owner)."""
    slot_win = []
    slot_q = []
    i = 0
    n = len(qidx)
    while i < n:
        w = wins[i]
        j = i
        while j < n and wins[j] == w:
            j += 1
        for s in range(i, j, CAP):
            slot_win.append(w)
            slot_q.append(qidx[s:min(s + CAP, j)])
        i = j
    assert len(slot_win) <= NSLOT, f"slot overflow: {len(slot_win)}"
    while len(slot_win) < NSLOT:
        slot_win.append(slot_win[0])
        slot_q.append(np.empty([0], np.int64))

    owner = np.full([NQ], -1, np.int64)
    xq = np.zeros([NQ, C], np.float32)
    for si, qs in enumerate(slot_q):
        if len(qs):
            xq[si * CAP: si * CAP + len(qs)] = x_b[qs]
            owner[si * CAP: si * CAP + len(qs)] = qs
    xqT = np.ascontiguousarray(xq.T).astype(ml_dtypes.bfloat16)
    xpT = np.ascontiguousarray(
        xp_b[np.asarray(slot_win)].reshape(NSLOT * K, C).T
    ).astype(ml_dtypes.bfloat16)
    return xqT, xpT, owner


def make_in_maps(x, x_permute, idx_win, Wq, bq, Wk, bk, Wv, bv, Wo, bo):
    x = np.asarray(x, np.float32)
    xp = np.asarray(x_permute, np.float32)
    idx = np.asarray(idx_win)
    shared = {
        "wqT": np.ascontiguousarray(np.asarray(Wq, np.float32).T).astype(ml_dtypes.bfloat16),
        "wkT": np.ascontiguousarray(np.asarray(Wk, np.float32).T).astype(ml_dtypes.bfloat16),
        "wvT": np.ascontiguousarray(np.asarray(Wv, np.float32).T).astype(ml_dtypes.bfloat16),
        "woT": np.ascontiguousarray(np.asarray(Wo, np.float32).T).astype(ml_dtypes.bfloat16),
        "bq": (np.asarray(bq, np.float32) * SCALE).astype(np.float32),
        "bk": np.asarray(bk, np.float32),
        "bv": np.asarray(bv, np.float32),
        "bo": np.asarray(bo, np.float32),
    }
    in_maps, owners = [], []
    for core in range(NCORES):
        b, qtr = divmod(core, QTRS)
        order = np.argsort(idx[b], kind="stable")
        qidx = order[qtr * NLOC:(qtr + 1) * NLOC]
        wins = idx[b][qidx]
        xqT, xpT, owner = _pack_core(x[b], xp[b], qidx, wins)
        in_maps.append({"xqT": xqT, "xpT": xpT, **shared})
        owners.append((b, owner))
    return in_maps, owners


def kernel(x, x_permute, idx_win, Wq, bq, Wk, bk, Wv, bv, Wo, bo):
    from concourse.bass_utils import run_bass_kernel_spmd

    nc = _get_program()
    in_maps, owners = make_in_maps(
        x, x_permute, idx_win, Wq, bq, Wk, bk, Wv, bv, Wo, bo)
    res = run_bass_kernel_spmd(nc, in_maps, list(range(NCORES)))
    out = np.zeros([B, N, C], np.float32)
    for core in range(NCORES):
        b, owner = owners[core]
        oT = np.asarray(res.results[core]["outT"], np.float32)
        valid = owner >= 0
        out[b][owner[valid]] = oT.T[valid]
    return out
